# revision 1
# baseline (speedup 1.0000x reference)
"""Trainium2 Bass kernel for CrossAttention3D (single-head, 512-dim, 4x64x64).

Math (per batch b, x = q[b] viewed (C, S)):
    qp = Wq x + bq ; kp = Wk x + bk ; vf = x^T Wv^T + bv
    simT = kp^T qp * C^-0.5 ; E = exp(simT) ; den = colsum(E)
    out = Wo (vf^T E) / den + bo

Sharding: 8 cores = 4 batches x 2 query-halves. Each core projects K/V for
its batch's full 4096 tokens, Q for its own 2048-query half, runs a fused
flash-style attention (no max subtraction: |sim| < 1.5), then the output
projection. Softmax normalization is applied AFTER the (linear) output
projection so the 1/den broadcast is off the PE critical path. No
collectives; host scatters/gathers.

On-chip layouts (partition dim first):
    kp  [128, 4, 4096] (o%128, o//128, s)  bf16 - lhsT for sim^T
    qp  [128, 4, 512]  (o%128, o//128, q)  bf16 - rhs for sim^T (scale folded)
    vf  [128, 32, 512] (s%128, s//128, c)  bf16 - lhsT for E^T@V
    sim^T psum [128(k), 512(q)] -> ACT exp -> et bf16 -> AV + ones-colsum MMs
    All PSUM evictions are on ACT so PE slot-reuse waits merge with data waits.
"""

import numpy as np
import ml_dtypes

import concourse.bass as bass
import concourse.bacc as bacc
import concourse.tile as tile
from concourse import mybir
from concourse.bass_utils import run_bass_kernel_spmd

AF = mybir.ActivationFunctionType
F32 = mybir.dt.float32
BF16 = mybir.dt.bfloat16

B, C, H, W = 4, 512, 64, 64
S = H * W            # 4096 tokens
P = 128              # partitions
CC = C // P          # 4 channel chunks
QH = S // 2          # 2048 queries per core
FB = 512             # free-dim block
NQB = QH // FB       # 4 query blocks per core
NSB = S // FB        # 8 token blocks
NKC = S // P         # 32 key chunks
N_CORES = 8


def _build_bass() -> bass.Bass:
    nc = bacc.Bacc("TRN2", target_bir_lowering=False)

    x_d = nc.dram_tensor("x", [C, S], BF16, kind="ExternalInput")
    xq_d = nc.dram_tensor("xq", [C, QH], BF16, kind="ExternalInput")
    wq_d = nc.dram_tensor("wq", [C, C], BF16, kind="ExternalInput")  # Wq.T * scale
    wk_d = nc.dram_tensor("wk", [C, C], BF16, kind="ExternalInput")  # Wk.T
    wv_d = nc.dram_tensor("wv", [C, C], BF16, kind="ExternalInput")  # Wv.T
    wo_d = nc.dram_tensor("wo", [C, C], BF16, kind="ExternalInput")  # Wo.T
    bq_d = nc.dram_tensor("bq", [C], F32, kind="ExternalInput")      # bq * scale
    bk_d = nc.dram_tensor("bk", [C], F32, kind="ExternalInput")
    bv_d = nc.dram_tensor("bv", [C], F32, kind="ExternalInput")
    bo_d = nc.dram_tensor("bo", [C], F32, kind="ExternalInput")
    out_d = nc.dram_tensor("out", [C, QH], F32, kind="ExternalOutput")

    x_r = x_d[:, :].rearrange("(cc p) s -> p cc s", p=P)
    xq_r = xq_d[:, :].rearrange("(cc p) s -> p cc s", p=P)
    out_r = out_d[:, :].rearrange("(oc p) s -> p oc s", p=P)

    def bcast_ap(ap_1d, parts):
        # [n] -> [parts, n] via 0-stride partition dim (DMA-only pattern)
        return bass.AP(
            tensor=ap_1d.tensor, offset=ap_1d.offset,
            ap=[[0, parts]] + [list(d) for d in ap_1d.ap],
        )

    with tile.TileContext(nc) as tc:
        with (
            tc.tile_pool(name="consts", bufs=1) as consts,
            tc.tile_pool(name="kv", bufs=1) as kv,
            tc.tile_pool(name="xs", bufs=3) as xs,
            tc.tile_pool(name="qps", bufs=2) as qps,
            tc.tile_pool(name="ets", bufs=6) as ets,
            tc.tile_pool(name="xus", bufs=6) as xus,
            tc.tile_pool(name="outs", bufs=4) as outs,
            tc.tile_pool(name="dscr", bufs=2, space="DRAM") as dscr,
            tc.tile_pool(name="psmm", bufs=3, space="PSUM") as psmm,
            tc.tile_pool(name="psav", bufs=4, space="PSUM") as psav,
            tc.tile_pool(name="psden", bufs=1, space="PSUM") as psden,
        ):
            # ---- constants; first x block + phase-1 weights lead so PE
            # ---- starts as early as possible, Wq/Wo deferred to phase 2 ----
            xt_first = xs.tile([P, CC, FB], BF16, tag="xt", name="xt_first")
            nc.sync.dma_start(out=xt_first, in_=x_r[:, :, 0:FB])
            wv_sb = consts.tile([P, CC, C], BF16, tag="wv")
            nc.sync.dma_start(out=wv_sb, in_=wv_d[:, :].rearrange("(cc p) o -> p cc o", p=P))
            wk_sb = consts.tile([P, CC, C], BF16, tag="wk")
            nc.sync.dma_start(out=wk_sb, in_=wk_d[:, :].rearrange("(cc p) o -> p cc o", p=P))
            bvb_sb = consts.tile([P, C], F32, tag="bvb")
            nc.sync.dma_start(out=bvb_sb, in_=bcast_ap(bv_d[:], P))
            bk_sb = consts.tile([P, CC], F32, tag="bk")
            nc.sync.dma_start(out=bk_sb, in_=bk_d[:].rearrange("(cc p) -> p cc", p=P))
            ones_col = consts.tile([P, 1], BF16, tag="ones_col")
            nc.vector.memset(ones_col, 1.0)

            # ---- persistent K / V ----
            kp_sb = kv.tile([P, CC, S], BF16, tag="kp")
            vf_sb = kv.tile([P, NKC, C], BF16, tag="vf")

            # ---- phase 1: K and V projections over the full sequence ----
            # V first: its Ldweights (lhsT = xt slice) absorbs the xt DMA wait
            # on PE, so the K matmuls that follow carry at most one wait.
            for sb in range(NSB):
                if sb == 0:
                    xt = xt_first
                else:
                    xt = xs.tile([P, CC, FB], BF16, tag="xt")
                    nc.sync.dma_start(out=xt, in_=x_r[:, :, sb * FB:(sb + 1) * FB])
                for i4 in range(CC):
                    sc = sb * CC + i4
                    psv = psav.tile([P, FB], F32, tag="av")
                    for cc in range(CC):
                        nc.tensor.matmul(
                            psv, lhsT=xt[:, cc, i4 * P:(i4 + 1) * P],
                            rhs=wv_sb[:, cc, :],
                            start=(cc == 0), stop=(cc == CC - 1),
                        )
                    nc.vector.tensor_add(out=vf_sb[:, sc, :], in0=psv[:], in1=bvb_sb[:])
                for oc in range(CC):
                    ps = psmm.tile([P, FB], F32, tag="mm")
                    for cc in range(CC):
                        nc.tensor.matmul(
                            ps, lhsT=wk_sb[:, cc, oc * P:(oc + 1) * P],
                            rhs=xt[:, cc, :],
                            start=(cc == 0), stop=(cc == CC - 1),
                        )
                    nc.scalar.activation(
                        out=kp_sb[:, oc, sb * FB:(sb + 1) * FB], in_=ps[:],
                        func=AF.Identity, bias=bk_sb[:, oc:oc + 1], scale=1.0,
                    )

            # ---- deferred constants for phase 2 ----
            wq_sb = consts.tile([P, CC, C], BF16, tag="wq")
            nc.sync.dma_start(out=wq_sb, in_=wq_d[:, :].rearrange("(cc p) o -> p cc o", p=P))
            wo_sb = consts.tile([P, CC, C], BF16, tag="wo")
            nc.sync.dma_start(out=wo_sb, in_=wo_d[:, :].rearrange("(cc p) o -> p cc o", p=P))
            bq_sb = consts.tile([P, CC], F32, tag="bq")
            nc.sync.dma_start(out=bq_sb, in_=bq_d[:].rearrange("(cc p) -> p cc", p=P))
            bo_sb = consts.tile([P, CC], F32, tag="bo")
            nc.sync.dma_start(out=bo_sb, in_=bo_d[:].rearrange("(cc p) -> p cc", p=P))

            # ---- phase 2: per query-block fused attention ----
            for qb in range(NQB):
                qsl = slice(qb * FB, (qb + 1) * FB)

                # Q projection (scale pre-folded into wq/bq)
                xqt = xs.tile([P, CC, FB], BF16, tag="xt")
                nc.sync.dma_start(out=xqt, in_=xq_r[:, :, qsl])
                qp = qps.tile([P, CC, FB], BF16, tag="qp")
                for oc in range(CC):
                    ps = psmm.tile([P, FB], F32, tag="mm")
                    for cc in range(CC):
                        nc.tensor.matmul(
                            ps, lhsT=wq_sb[:, cc, oc * P:(oc + 1) * P],
                            rhs=xqt[:, cc, :],
                            start=(cc == 0), stop=(cc == CC - 1),
                        )
                    nc.scalar.activation(
                        out=qp[:, oc, :], in_=ps[:],
                        func=AF.Identity, bias=bq_sb[:, oc:oc + 1], scale=1.0,
                    )

                den = psden.tile([1, FB], F32, tag="den")
                avt = [psav.tile([P, FB], F32, tag="av", name=f"avt{qb}_{i}") for i in range(CC)]
                for kc in range(NKC):
                    simt = psmm.tile([P, FB], F32, tag="mm")
                    for oc in range(CC):
                        nc.tensor.matmul(
                            simt, lhsT=kp_sb[:, oc, kc * P:(kc + 1) * P],
                            rhs=qp[:, oc, :],
                            start=(oc == 0), stop=(oc == CC - 1),
                        )
                    et = ets.tile([P, FB], BF16, tag="et")
                    nc.scalar.activation(out=et, in_=simt[:], func=AF.Exp)
                    nc.tensor.matmul(
                        den, lhsT=ones_col[:], rhs=et[:],
                        start=(kc == 0), stop=(kc == NKC - 1),
                    )
                    for c4 in range(CC):
                        nc.tensor.matmul(
                            avt[c4], lhsT=vf_sb[:, kc, c4 * P:(c4 + 1) * P],
                            rhs=et[:],
                            start=(kc == 0), stop=(kc == NKC - 1),
                        )

                # 1/den, broadcast to all partitions via a DRAM bounce; runs
                # concurrently with the output-projection matmuls below.
                den_sb = xs.tile([1, FB], F32, tag="den_sb")
                nc.scalar.activation(out=den_sb, in_=den[:], func=AF.Copy)
                rec = xs.tile([1, FB], F32, tag="rec")
                nc.vector.reciprocal(out=rec, in_=den_sb[:])
                rscr = dscr.tile([1, FB], F32, tag="rscr")
                nc.sync.dma_start(out=rscr, in_=rec[:])
                rbc = xs.tile([P, FB], F32, tag="rbc")
                nc.sync.dma_start(out=rbc, in_=bcast_ap(rscr[0, :], P))

                # evict unnormalized attention output (ACT keeps slot waits
                # mergeable on PE), then project
                xu = [None] * CC
                for c4 in range(CC):
                    xu[c4] = xus.tile([P, FB], BF16, tag="xu", name=f"xu{qb}_{c4}")
                    nc.scalar.activation(out=xu[c4], in_=avt[c4][:], func=AF.Copy)

                for oc in range(CC):
                    po = psmm.tile([P, FB], F32, tag="mm")
                    for c4 in range(CC):
                        nc.tensor.matmul(
                            po, lhsT=wo_sb[:, c4, oc * P:(oc + 1) * P],
                            rhs=xu[c4][:],
                            start=(c4 == 0), stop=(c4 == CC - 1),
                        )
                    yo = outs.tile([P, FB], F32, tag="yo")
                    nc.scalar.activation(out=yo, in_=po[:], func=AF.Copy)
                    # out = yo/den + bo  (normalization commutes with Wo)
                    ot = outs.tile([P, FB], F32, tag="ot")
                    nc.vector.tensor_mul(out=ot, in0=yo[:], in1=rbc[:])
                    nc.vector.tensor_scalar_add(out=ot, in0=ot[:], scalar1=bo_sb[:, oc:oc + 1])
                    nc.sync.dma_start(out=out_r[:, oc, qsl], in_=ot[:])

    nc.finalize()
    return nc


_NC_CACHE = {}


def _get_nc() -> bass.Bass:
    if "nc" not in _NC_CACHE:
        _NC_CACHE["nc"] = _build_bass()
    return _NC_CACHE["nc"]


def make_in_maps(q, Wq, bq, Wk, bk, Wv, bv, Wo, bo):
    f = np.float32
    bf = ml_dtypes.bfloat16
    scale = f(C) ** f(-0.5)
    wq = np.ascontiguousarray((np.asarray(Wq, f).T * scale).astype(bf))
    wk = np.ascontiguousarray(np.asarray(Wk, f).T.astype(bf))
    wv = np.ascontiguousarray(np.asarray(Wv, f).T.astype(bf))
    wo = np.ascontiguousarray(np.asarray(Wo, f).T.astype(bf))
    bqs = np.asarray(bq, f) * scale
    bk = np.asarray(bk, f)
    bv = np.asarray(bv, f)
    bo = np.asarray(bo, f)
    in_maps = []
    for core in range(N_CORES):
        b, half = core // 2, core % 2
        x = np.asarray(q[b], f).reshape(C, S).astype(bf)
        xq = np.ascontiguousarray(x[:, half * QH:(half + 1) * QH])
        in_maps.append({
            "x": np.ascontiguousarray(x), "xq": xq,
            "wq": wq, "wk": wk, "wv": wv, "wo": wo,
            "bq": bqs, "bk": bk, "bv": bv, "bo": bo,
        })
    return in_maps


def gather_out(per_core_outs):
    out = np.zeros((B, C, S), np.float32)
    for core in range(N_CORES):
        b, half = core // 2, core % 2
        out[b, :, half * QH:(half + 1) * QH] = per_core_outs[core]
    return out.reshape(B, C, H, W)


def kernel(q, Wq, bq, Wk, bk, Wv, bv, Wo, bo):
    nc = _get_nc()
    in_maps = make_in_maps(q, Wq, bq, Wk, bk, Wv, bv, Wo, bo)
    res = run_bass_kernel_spmd(nc, in_maps, core_ids=list(range(N_CORES)))
    return gather_out([res.results[i]["out"] for i in range(N_CORES)])



# revision 2
# speedup vs baseline: 1.1497x; 1.1497x over previous
"""Trainium2 Bass kernel for CrossAttention3D (single-head, 512-dim, 4x64x64).

Algebraic restructure (per batch, X = q[b] viewed (C, S)):
    Softmax logits are invariant to per-query additive constants, so with
    qp = Wq X + bq, kp = Wk X + bk:
        logits[q,k] = scale * qp_q . kp_k
                    = (scale Wk^T Wq X_q) . X_k + beta_k  (+ per-q const, dropped)
        beta_k      = scale * (Wk^T bq) . X_k             (+ const, dropped)
    and the V/O projections commute through the (linear) attention average:
        out = (Wo Wv)(X E)/den + (Wo bv + bo),  E = exp(logits), den = 1^T E.
    Only TWO device projections remain: the fused QK projection applied on
    the query side (2048 cols/core, not 4096) and the fused VO projection
    after attention. beta rides along as the exp's per-partition bias.

All heavy matmuls are fp8 e4m3 with DoubleRow perf mode (two 128-deep
k-tiles contracted per instruction; measured ~259ns per 512-row matmul =
2x bf16 FLOP rate, LdWeights fully hidden). Rescales keep operands in
e4m3's normal range: M0' = 32*scale*Wk^T Wq, N0' = 32*Wo Wv, exp scale=1/32,
XE evicted with scale 1/16, den "ones" = 2.0, so 32*(1/16)/2 = 1 net.

Schedule: per query block, sim leads and den/AV consume et one iteration
behind so the PE never waits on the exp. The denominator accumulates on the
DVE for iterations 0..13 (evicted to bf16 off the critical path) and the
last two et pairs close it directly in PSUM via DR matmuls. 1/den (approx
reciprocal on DVE) is broadcast to all partitions with a rank-1 f32 matmul
into PSUM -- no DRAM bounce -- and the VO-projection PSUM is evicted to
SBUF immediately so no PE instruction ever waits on the normalization.
Steady state measures ~216ns per matmul issue: the PE runs at ~99% of its
fp8 peak inside the attention loop.

Sharding: 8 cores = 4 batches x 2 query-halves, no collectives.
"""

import numpy as np
import ml_dtypes

import concourse.bass as bass
import concourse.bacc as bacc
import concourse.tile as tile
from concourse import mybir
from concourse.bass_utils import run_bass_kernel_spmd

AF = mybir.ActivationFunctionType
DR = mybir.MatmulPerfMode.DoubleRow
F32 = mybir.dt.float32
BF16 = mybir.dt.bfloat16
F8 = mybir.dt.float8e4

B, C, H, W = 4, 512, 64, 64
S = H * W            # 4096 tokens
P = 128              # partitions
CC = C // P          # 4 channel chunks
QH = S // 2          # 2048 queries per core
FB = 512             # query block
NQB = QH // FB       # 4 query blocks per core
NKC = S // P         # 32 key chunks
NI = NKC // 2        # 16 key-pair iterations
N_CORES = 8


def _build_bass() -> bass.Bass:
    nc = bacc.Bacc("TRN2", target_bir_lowering=False)

    x_d = nc.dram_tensor("x", [C, S], F8, kind="ExternalInput")     # keys, fp8
    xq_d = nc.dram_tensor("xq", [C, QH], F8, kind="ExternalInput")  # this core's queries
    xt_d = nc.dram_tensor("xt", [S, C], F8, kind="ExternalInput")   # X^T, fp8
    m0_d = nc.dram_tensor("m0", [C, C], F8, kind="ExternalInput")   # (32*scale*Wk^T Wq)^T
    n0_d = nc.dram_tensor("n0", [C, C], F8, kind="ExternalInput")   # (32*Wo@Wv)^T
    # all 2.0; padded to stride 16 to satisfy dual-fp8 Ldweights alignment
    ones_d = nc.dram_tensor("ones2", [P, 32], F8, kind="ExternalInput")
    beta_d = nc.dram_tensor("beta", [S], F32, kind="ExternalInput")
    b2_d = nc.dram_tensor("b2", [C], F32, kind="ExternalInput")
    out_d = nc.dram_tensor("out", [C, QH], F32, kind="ExternalOutput")

    x_r = x_d[:, :].rearrange("(cc p) s -> p cc s", p=P)
    xq_r = xq_d[:, :].rearrange("(cc p) s -> p cc s", p=P)
    xt_r = xt_d[:, :].rearrange("(kc p) c -> p kc c", p=P)
    out_r = out_d[:, :].rearrange("(oc p) s -> p oc s", p=P)

    with tile.TileContext(nc) as tc:
        with (
            tc.tile_pool(name="consts", bufs=1) as consts,
            tc.tile_pool(name="kkqs", bufs=2) as kkqs,
            tc.tile_pool(name="ets", bufs=6) as ets,
            tc.tile_pool(name="esums", bufs=2) as esums,
            tc.tile_pool(name="xes", bufs=2) as xes,
            tc.tile_pool(name="smalls", bufs=4) as smalls,
            tc.tile_pool(name="outs", bufs=8) as outs,
            tc.tile_pool(name="psmm", bufs=4, space="PSUM") as psmm,   # 4 banks
            tc.tile_pool(name="psav", bufs=4, space="PSUM") as psav,   # 4 banks
        ):
            # ---- constants. x8/xt8/xq8 are split into per-chunk tiles so
            # ---- their DMAs stream without WAW completion waits; keys ride
            # ---- the sync queue, the transpose rides the gpsimd SWDGE
            # ---- queue, and ACT issues no DMAs (they would block evicts) ----
            m0_sb = consts.tile([P, CC, C], F8, tag="m0")
            nc.sync.dma_start(out=m0_sb, in_=m0_d[:, :].rearrange("(cc p) o -> p cc o", p=P))
            xq8_t = [consts.tile([P, CC, FB], F8, tag=f"xq8_{i}", name=f"xq8_{i}")
                     for i in range(NQB)]
            nc.sync.dma_start(out=xq8_t[0], in_=xq_r[:, :, 0:FB])
            ones_sb = consts.tile([P, 1], BF16, tag="ones")
            nc.vector.memset(ones_sb, 2.0)
            ones8_sb = consts.tile([P, 2, 16], F8, tag="ones8")
            nc.sync.dma_start(out=ones8_sb, in_=ones_d[:, :])
            ones1_sb = consts.tile([1, P], F32, tag="ones1")
            nc.vector.memset(ones1_sb, 1.0)
            SQ = S // 4
            x8_t = [consts.tile([P, CC, SQ], F8, tag=f"x8_{i}", name=f"x8_{i}")
                    for i in range(4)]
            xt8_t = [consts.tile([P, NKC // 4, C], F8, tag=f"xt8_{i}", name=f"xt8_{i}")
                     for i in range(4)]
            nc.sync.dma_start(out=x8_t[0], in_=x_r[:, :, 0:SQ])
            beta_sb = consts.tile([P, NKC], F32, tag="beta")
            nc.sync.dma_start(out=beta_sb, in_=beta_d[:].rearrange("(kc p) -> p kc", p=P))
            nc.sync.dma_start(out=xt8_t[0], in_=xt_r[:, 0:NKC // 4, :])
            for c4 in range(1, 4):
                nc.sync.dma_start(out=x8_t[c4], in_=x_r[:, :, c4 * SQ:(c4 + 1) * SQ])
                klo = c4 * (NKC // 4)
                nc.sync.dma_start(out=xt8_t[c4], in_=xt_r[:, klo:klo + NKC // 4, :])
            for qb in range(1, NQB):
                nc.sync.dma_start(out=xq8_t[qb], in_=xq_r[:, :, qb * FB:(qb + 1) * FB])
            n0_sb = consts.tile([P, CC, C], F8, tag="n0")
            nc.sync.dma_start(out=n0_sb, in_=n0_d[:, :].rearrange("(cc p) o -> p cc o", p=P))
            b2_sb = consts.tile([P, CC], F32, tag="b2")
            nc.sync.dma_start(out=b2_sb, in_=b2_d[:].rearrange("(oc p) -> p oc", p=P))

            def x8_key(cc, k0):
                # sim lhsT [128, 2(cc pair), P] for keys [k0, k0+P)
                t = x8_t[k0 // SQ]
                off = k0 % SQ
                return t[:, cc:cc + 2, off:off + P]

            def xt8_key(kc, c4):
                # AV lhsT [128, 2(kc pair), P] for channel chunk c4
                t = xt8_t[kc // 8]
                return t[:, kc % 8:kc % 8 + 2, c4 * P:(c4 + 1) * P]

            kkq_tiles = {}

            def emit_qproj(qb):
                # fused QK projection of query block qb (PE + ACT evict)
                kkq = kkqs.tile([P, CC, FB], F8, tag="kkq", name=f"kkq{qb}")
                for oc in range(CC):
                    ps = psav.tile([P, FB], F32, tag="av")
                    for ci, cc in enumerate(range(0, CC, 2)):
                        nc.tensor.matmul(
                            ps, lhsT=m0_sb[:, cc:cc + 2, oc * P:(oc + 1) * P],
                            rhs=xq8_t[qb][:, cc:cc + 2, :],
                            start=(ci == 0), stop=(ci == 1), perf_mode=DR,
                        )
                    nc.scalar.activation(out=kkq[:, oc, :], in_=ps[:], func=AF.Copy)
                kkq_tiles[qb] = kkq

            emit_qproj(0)

            for qb in range(NQB):
                qsl = slice(qb * FB, (qb + 1) * FB)
                kkq = kkq_tiles[qb]

                # ---- attention: sim leads; esum/AV consume et one iter behind ----
                avt = [psav.tile([P, FB], F32, tag="av", name=f"avt{qb}_{i}")
                       for i in range(CC)]
                esum = esums.tile([P, 2, FB], F32, tag="esum")
                ets_q = []

                def emit_sim(i, qb=qb, kkq=kkq):
                    simt = [psmm.tile([P, FB], F32, tag="mm", name=f"sim{qb}_{i}_{j}")
                            for j in range(2)]
                    for j in range(2):
                        k0 = (2 * i + j) * P
                        for ci, cc in enumerate(range(0, CC, 2)):
                            nc.tensor.matmul(
                                simt[j], lhsT=x8_key(cc, k0),
                                rhs=kkq[:, cc:cc + 2, :],
                                start=(ci == 0), stop=(ci == 1), perf_mode=DR,
                            )
                    et = ets.tile([P, 2, FB], F8, tag="et", name=f"et{qb}_{i}")
                    for j in range(2):
                        kc = 2 * i + j
                        nc.scalar.activation(
                            out=et[:, j, :], in_=simt[j][:], func=AF.Exp,
                            bias=beta_sb[:, kc:kc + 1], scale=1.0 / 32.0,
                        )
                    ets_q.append(et)

                den = [None]

                def emit_denav(i, qb=qb, avt=avt, esum=esum, ets_q=ets_q):
                    et = ets_q[i]
                    # denominator partials: iters 0..13 accumulate on the DVE
                    # (off the critical path); the last two go straight into
                    # the den PSUM with DR matmuls so den closes ~1us after
                    # the final exp
                    if i == 0:
                        nc.vector.tensor_scalar_add(out=esum, in0=et[:, :, :], scalar1=0.0)
                    elif i <= NI - 3:
                        nc.vector.tensor_add(out=esum, in0=esum[:, :, :], in1=et[:, :, :])
                    else:
                        if i == NI - 2:
                            den[0] = psmm.tile([1, FB], F32, tag="mm", name=f"den{qb}")
                        nc.tensor.matmul(
                            den[0], lhsT=ones8_sb[:, :, 0:1], rhs=et[:, :, :],
                            start=(i == NI - 2), stop=False, perf_mode=DR,
                        )
                    for c4 in range(CC):
                        nc.tensor.matmul(
                            avt[c4], lhsT=xt8_key(2 * i, c4),
                            rhs=et[:, :, :],
                            start=(i == 0), stop=(i == NI - 1), perf_mode=DR,
                        )

                for i in range(NI):
                    emit_sim(i)
                    if i > 0:
                        emit_denav(i - 1)
                    if i == NI - 1:
                        # esbA (iters 0..13) evicts early, off the tail path
                        esb = smalls.tile([P, 2, FB], BF16, tag="esb")
                        nc.scalar.activation(out=esb, in_=esum[:, :, :], func=AF.Copy)
                # last iteration: AV matmuls + den DR matmul, then XE evicts
                et15 = ets_q[NI - 1]
                for c4 in range(CC):
                    nc.tensor.matmul(
                        avt[c4], lhsT=xt8_key(2 * (NI - 1), c4),
                        rhs=et15[:, :, :],
                        start=False, stop=True, perf_mode=DR,
                    )
                nc.tensor.matmul(
                    den[0], lhsT=ones8_sb[:, :, 0:1], rhs=et15[:, :, :],
                    start=False, stop=False, perf_mode=DR,
                )
                nc.tensor.matmul(den[0], lhsT=ones_sb[:], rhs=esb[:, 0, :], start=False, stop=False)
                nc.tensor.matmul(den[0], lhsT=ones_sb[:], rhs=esb[:, 1, :], start=False, stop=True)
                xe8 = xes.tile([P, CC, FB], F8, tag="xe8")
                for c4 in range(CC):
                    nc.vector.tensor_scalar_mul(
                        out=xe8[:, c4, :], in0=avt[c4][:], scalar1=1.0 / 16.0)

                # ---- boundary. ACT: kkq evicts, po evicts; DVE: XE, rec,
                # ---- final scale+bias; PE: qproj, VO -- so no PE
                # ---- instruction ever waits on the rbc bounce ----
                # next block's projection fills the PE while rbc is in flight
                if qb + 1 < NQB:
                    emit_qproj(qb + 1)

                rec = smalls.tile([1, FB], F32, tag="rec")
                nc.vector.reciprocal_approx_fast(out=rec, in_=den[0][:])
                rbc = psmm.tile([P, FB], F32, tag="mm", name=f"rbc{qb}")
                nc.tensor.matmul(
                    rbc, lhsT=ones1_sb[:], rhs=rec[:],
                    start=True, stop=True,
                )

                # ---- fused VO projection; po evicted to SBUF immediately so
                # ---- the PSUM slot never waits on the rbc broadcast ----
                for oc in range(CC):
                    po = psmm.tile([P, FB], F32, tag="mm", name=f"po{qb}_{oc}")
                    for ci, cc in enumerate(range(0, CC, 2)):
                        nc.tensor.matmul(
                            po, lhsT=n0_sb[:, cc:cc + 2, oc * P:(oc + 1) * P],
                            rhs=xe8[:, cc:cc + 2, :],
                            start=(ci == 0), stop=(ci == 1), perf_mode=DR,
                        )
                    yo = outs.tile([P, FB], F32, tag="yo", name=f"yo{qb}_{oc}")
                    nc.scalar.activation(out=yo, in_=po[:], func=AF.Copy)
                    ot = outs.tile([P, FB], F32, tag="ot")
                    nc.vector.tensor_mul(out=ot, in0=yo[:], in1=rbc[:])
                    nc.vector.tensor_scalar_add(out=ot, in0=ot[:], scalar1=b2_sb[:, oc:oc + 1])
                    nc.sync.dma_start(out=out_r[:, oc, qsl], in_=ot[:])

    nc.finalize()
    return nc


_NC_CACHE = {}


def _get_nc() -> bass.Bass:
    if "nc" not in _NC_CACHE:
        _NC_CACHE["nc"] = _build_bass()
    return _NC_CACHE["nc"]


def make_in_maps(q, Wq, bq, Wk, bk, Wv, bv, Wo, bo):
    f = np.float32
    f8 = ml_dtypes.float8_e4m3
    scale = f(C) ** f(-0.5)

    def q8(a):
        return np.ascontiguousarray(
            np.clip(np.asarray(a, f), -240, 240).astype(f8))

    Wq, Wk, Wv, Wo = (np.asarray(a, f) for a in (Wq, Wk, Wv, Wo))
    bq, bk, bv, bo = (np.asarray(a, f) for a in (bq, bk, bv, bo))
    m0 = q8(((Wk.T @ Wq) * (32.0 * scale)).T)   # dram[c, o] = M0'[o, c]
    n0 = q8((32.0 * (Wo @ Wv)).T)
    ones2 = np.full((P, 32), 2.0, f8)
    w_beta = (Wk.T @ bq) * scale
    b2 = (Wo @ bv + bo).astype(f)

    in_maps = []
    for core in range(N_CORES):
        b, half = core // 2, core % 2
        X = np.asarray(q[b], f).reshape(C, S)
        x8 = q8(X)
        in_maps.append({
            "x": x8,
            "xq": np.ascontiguousarray(x8[:, half * QH:(half + 1) * QH]),
            "xt": np.ascontiguousarray(x8.T),
            "m0": m0, "n0": n0, "ones2": ones2,
            "beta": (w_beta @ X).astype(f), "b2": b2,
        })
    return in_maps


def gather_out(per_core_outs):
    out = np.zeros((B, C, S), np.float32)
    for core in range(N_CORES):
        b, half = core // 2, core % 2
        out[b, :, half * QH:(half + 1) * QH] = per_core_outs[core]
    return out.reshape(B, C, H, W)


def kernel(q, Wq, bq, Wk, bk, Wv, bv, Wo, bo):
    nc = _get_nc()
    in_maps = make_in_maps(q, Wq, bq, Wk, bk, Wv, bv, Wo, bo)
    res = run_bass_kernel_spmd(nc, in_maps, core_ids=list(range(N_CORES)))
    return gather_out([res.results[i]["out"] for i in range(N_CORES)])


# revision 3
# speedup vs baseline: 1.1548x; 1.0044x over previous
"""Trainium2 Bass kernel for CrossAttention3D (single-head, 512-dim, 4x64x64).

Algebraic restructure (per batch, X = q[b] viewed (C, S)):
    Softmax logits are invariant to per-query additive constants, so with
    qp = Wq X + bq, kp = Wk X + bk:
        logits[q,k] = scale * qp_q . kp_k
                    = (scale Wk^T Wq X_q) . X_k + beta_k  (+ per-q const, dropped)
        beta_k      = scale * (Wk^T bq) . X_k             (+ const, dropped)
    and the V/O projections commute through the (linear) attention average:
        out = (Wo Wv)(X E)/den + (Wo bv + bo),  E = exp(logits), den = 1^T E.
    Only TWO device projections remain: the fused QK projection applied on
    the query side (2048 cols/core, not 4096) and the fused VO projection
    after attention. beta rides along as the exp's per-partition bias.

All heavy matmuls are fp8 e4m3 with DoubleRow perf mode (two 128-deep
k-tiles contracted per instruction; measured ~259ns per 512-row matmul =
2x bf16 FLOP rate, LdWeights fully hidden). Rescales keep operands in
e4m3's normal range: M0' = 32*scale*Wk^T Wq, N0' = 32*Wo Wv, exp scale=1/32,
XE evicted with scale 1/16, den "ones" = 2.0, so 32*(1/16)/2 = 1 net.

Schedule: per query block, sim leads and den/AV consume et one iteration
behind so the PE never waits on the exp. The denominator accumulates on the
DVE for iterations 0..13 (evicted to bf16 off the critical path) and the
last two et pairs close it directly in PSUM via DR matmuls. 1/den (approx
reciprocal on DVE) is broadcast to all partitions with a rank-1 f32 matmul
into PSUM -- no DRAM bounce -- and the VO-projection PSUM is evicted to
SBUF immediately so no PE instruction ever waits on the normalization.
Steady state measures ~216ns per matmul issue: the PE runs at ~99% of its
fp8 peak inside the attention loop.

Sharding: 8 cores = 4 batches x 2 query-halves, no collectives.
"""

import numpy as np
import ml_dtypes

import concourse.bass as bass
import concourse.bacc as bacc
import concourse.tile as tile
from concourse import mybir
from concourse.bass_utils import run_bass_kernel_spmd

AF = mybir.ActivationFunctionType
DR = mybir.MatmulPerfMode.DoubleRow
F32 = mybir.dt.float32
BF16 = mybir.dt.bfloat16
F8 = mybir.dt.float8e4

B, C, H, W = 4, 512, 64, 64
S = H * W            # 4096 tokens
P = 128              # partitions
CC = C // P          # 4 channel chunks
QH = S // 2          # 2048 queries per core
FB = 512             # query block
NQB = QH // FB       # 4 query blocks per core
NKC = S // P         # 32 key chunks
NI = NKC // 2        # 16 key-pair iterations
N_CORES = 8


def _build_bass() -> bass.Bass:
    nc = bacc.Bacc("TRN2", target_bir_lowering=False)

    x_d = nc.dram_tensor("x", [C, S], F8, kind="ExternalInput")     # keys, fp8
    xq_d = nc.dram_tensor("xq", [C, QH], F8, kind="ExternalInput")  # this core's queries
    xt_d = nc.dram_tensor("xt", [S, C], F8, kind="ExternalInput")   # X^T, fp8
    # (32*scale*Wk^T Wq) pre-chunked host-side: [oc, c, o'] contiguous per oc
    m0_d = nc.dram_tensor("m0", [CC, C, P], F8, kind="ExternalInput")
    n0_d = nc.dram_tensor("n0", [C, C], F8, kind="ExternalInput")   # (32*Wo@Wv)^T
    # all 2.0; padded to stride 16 to satisfy dual-fp8 Ldweights alignment
    ones_d = nc.dram_tensor("ones2", [P, 32], F8, kind="ExternalInput")
    beta_d = nc.dram_tensor("beta", [S], F32, kind="ExternalInput")
    b2_d = nc.dram_tensor("b2", [C], F32, kind="ExternalInput")
    out_d = nc.dram_tensor("out", [C, QH], F32, kind="ExternalOutput")

    x_r = x_d[:, :].rearrange("(cc p) s -> p cc s", p=P)
    xq_r = xq_d[:, :].rearrange("(cc p) s -> p cc s", p=P)
    xt_r = xt_d[:, :].rearrange("(kc p) c -> p kc c", p=P)
    out_r = out_d[:, :].rearrange("(oc p) s -> p oc s", p=P)

    with tile.TileContext(nc) as tc:
        with (
            tc.tile_pool(name="consts", bufs=1) as consts,
            tc.tile_pool(name="kkqs", bufs=2) as kkqs,
            tc.tile_pool(name="ets", bufs=6) as ets,
            tc.tile_pool(name="esums", bufs=2) as esums,
            tc.tile_pool(name="xes", bufs=2) as xes,
            tc.tile_pool(name="smalls", bufs=4) as smalls,
            tc.tile_pool(name="outs", bufs=8) as outs,
            tc.tile_pool(name="psmm", bufs=4, space="PSUM") as psmm,   # 4 banks
            tc.tile_pool(name="psav", bufs=4, space="PSUM") as psav,   # 4 banks
        ):
            # ---- constants. x8/xt8/xq8 are split into per-chunk tiles so
            # ---- their DMAs stream without WAW completion waits; keys ride
            # ---- the sync queue, the transpose rides the gpsimd SWDGE
            # ---- queue, and ACT issues no DMAs (they would block evicts) ----
            m0_t = [consts.tile([P, CC, P], F8, tag=f"m0_{oc}", name=f"m0_{oc}")
                    for oc in range(CC)]
            nc.sync.dma_start(
                out=m0_t[0], in_=m0_d[0, :, :].rearrange("(cc p) o -> p cc o", p=P))
            xq8_t = [consts.tile([P, CC, FB], F8, tag=f"xq8_{i}", name=f"xq8_{i}")
                     for i in range(NQB)]
            nc.sync.dma_start(out=xq8_t[0], in_=xq_r[:, :, 0:FB])
            for oc in range(1, CC):
                nc.sync.dma_start(
                    out=m0_t[oc], in_=m0_d[oc, :, :].rearrange("(cc p) o -> p cc o", p=P))
            ones_sb = consts.tile([P, 1], BF16, tag="ones")
            nc.vector.memset(ones_sb, 2.0)
            ones8_sb = consts.tile([P, 2, 16], F8, tag="ones8")
            nc.sync.dma_start(out=ones8_sb, in_=ones_d[:, :])
            ones1_sb = consts.tile([1, P], F32, tag="ones1")
            nc.vector.memset(ones1_sb, 1.0)
            SQ = S // 4
            x8_t = [consts.tile([P, CC, SQ], F8, tag=f"x8_{i}", name=f"x8_{i}")
                    for i in range(4)]
            xt8_t = [consts.tile([P, NKC // 4, C], F8, tag=f"xt8_{i}", name=f"xt8_{i}")
                     for i in range(4)]
            nc.sync.dma_start(out=x8_t[0], in_=x_r[:, :, 0:SQ])
            beta_sb = consts.tile([P, NKC], F32, tag="beta")
            nc.sync.dma_start(out=beta_sb, in_=beta_d[:].rearrange("(kc p) -> p kc", p=P))
            nc.sync.dma_start(out=xt8_t[0], in_=xt_r[:, 0:NKC // 4, :])
            for c4 in range(1, 4):
                nc.sync.dma_start(out=x8_t[c4], in_=x_r[:, :, c4 * SQ:(c4 + 1) * SQ])
                klo = c4 * (NKC // 4)
                nc.sync.dma_start(out=xt8_t[c4], in_=xt_r[:, klo:klo + NKC // 4, :])
            for qb in range(1, NQB):
                nc.sync.dma_start(out=xq8_t[qb], in_=xq_r[:, :, qb * FB:(qb + 1) * FB])
            n0_sb = consts.tile([P, CC, C], F8, tag="n0")
            nc.sync.dma_start(out=n0_sb, in_=n0_d[:, :].rearrange("(cc p) o -> p cc o", p=P))
            b2_sb = consts.tile([P, CC], F32, tag="b2")
            nc.sync.dma_start(out=b2_sb, in_=b2_d[:].rearrange("(oc p) -> p oc", p=P))

            def x8_key(cc, k0):
                # sim lhsT [128, 2(cc pair), P] for keys [k0, k0+P)
                t = x8_t[k0 // SQ]
                off = k0 % SQ
                return t[:, cc:cc + 2, off:off + P]

            def xt8_key(kc, c4):
                # AV lhsT [128, 2(kc pair), P] for channel chunk c4
                t = xt8_t[kc // 8]
                return t[:, kc % 8:kc % 8 + 2, c4 * P:(c4 + 1) * P]

            kkq_tiles = {}

            def emit_qproj(qb):
                # fused QK projection of query block qb (PE + ACT evict)
                kkq = kkqs.tile([P, CC, FB], F8, tag="kkq", name=f"kkq{qb}")
                for oc in range(CC):
                    ps = psav.tile([P, FB], F32, tag="av")
                    for ci, cc in enumerate(range(0, CC, 2)):
                        nc.tensor.matmul(
                            ps, lhsT=m0_t[oc][:, cc:cc + 2, :],
                            rhs=xq8_t[qb][:, cc:cc + 2, :],
                            start=(ci == 0), stop=(ci == 1), perf_mode=DR,
                        )
                    nc.scalar.activation(out=kkq[:, oc, :], in_=ps[:], func=AF.Copy)
                kkq_tiles[qb] = kkq

            emit_qproj(0)

            for qb in range(NQB):
                qsl = slice(qb * FB, (qb + 1) * FB)
                kkq = kkq_tiles[qb]

                # ---- attention: sim leads; esum/AV consume et one iter behind ----
                avt = [psav.tile([P, FB], F32, tag="av", name=f"avt{qb}_{i}")
                       for i in range(CC)]
                esum = esums.tile([P, 2, FB], F32, tag="esum")
                ets_q = []

                def emit_sim(i, qb=qb, kkq=kkq):
                    simt = [psmm.tile([P, FB], F32, tag="mm", name=f"sim{qb}_{i}_{j}")
                            for j in range(2)]
                    for j in range(2):
                        k0 = (2 * i + j) * P
                        for ci, cc in enumerate(range(0, CC, 2)):
                            nc.tensor.matmul(
                                simt[j], lhsT=x8_key(cc, k0),
                                rhs=kkq[:, cc:cc + 2, :],
                                start=(ci == 0), stop=(ci == 1), perf_mode=DR,
                            )
                    et = ets.tile([P, 2, FB], F8, tag="et", name=f"et{qb}_{i}")
                    for j in range(2):
                        kc = 2 * i + j
                        nc.scalar.activation(
                            out=et[:, j, :], in_=simt[j][:], func=AF.Exp,
                            bias=beta_sb[:, kc:kc + 1], scale=1.0 / 32.0,
                        )
                    ets_q.append(et)

                den = [None]

                def emit_denav(i, qb=qb, avt=avt, esum=esum, ets_q=ets_q):
                    et = ets_q[i]
                    # denominator partials: iters 0..13 accumulate on the DVE
                    # (off the critical path); the last two go straight into
                    # the den PSUM with DR matmuls so den closes ~1us after
                    # the final exp
                    if i == 0:
                        nc.vector.tensor_scalar_add(out=esum, in0=et[:, :, :], scalar1=0.0)
                    elif i <= NI - 3:
                        nc.vector.tensor_add(out=esum, in0=esum[:, :, :], in1=et[:, :, :])
                    else:
                        if i == NI - 2:
                            den[0] = psmm.tile([1, FB], F32, tag="mm", name=f"den{qb}")
                        nc.tensor.matmul(
                            den[0], lhsT=ones8_sb[:, :, 0:1], rhs=et[:, :, :],
                            start=(i == NI - 2), stop=False, perf_mode=DR,
                        )
                    for c4 in range(CC):
                        nc.tensor.matmul(
                            avt[c4], lhsT=xt8_key(2 * i, c4),
                            rhs=et[:, :, :],
                            start=(i == 0), stop=(i == NI - 1), perf_mode=DR,
                        )

                for i in range(NI):
                    emit_sim(i)
                    if i > 0:
                        emit_denav(i - 1)
                    if i == NI - 1:
                        # esbA (iters 0..13) evicts early, off the tail path
                        esb = smalls.tile([P, 2, FB], BF16, tag="esb")
                        nc.scalar.activation(out=esb, in_=esum[:, :, :], func=AF.Copy)
                # last iteration: AV matmuls + den DR matmul, then XE evicts
                et15 = ets_q[NI - 1]
                for c4 in range(CC):
                    nc.tensor.matmul(
                        avt[c4], lhsT=xt8_key(2 * (NI - 1), c4),
                        rhs=et15[:, :, :],
                        start=False, stop=True, perf_mode=DR,
                    )
                nc.tensor.matmul(
                    den[0], lhsT=ones8_sb[:, :, 0:1], rhs=et15[:, :, :],
                    start=False, stop=False, perf_mode=DR,
                )
                nc.tensor.matmul(den[0], lhsT=ones_sb[:], rhs=esb[:, 0, :], start=False, stop=False)
                nc.tensor.matmul(den[0], lhsT=ones_sb[:], rhs=esb[:, 1, :], start=False, stop=True)
                xe8 = xes.tile([P, CC, FB], F8, tag="xe8")
                for c4 in range(CC):
                    nc.vector.tensor_scalar_mul(
                        out=xe8[:, c4, :], in0=avt[c4][:], scalar1=1.0 / 16.0)

                # ---- boundary. ACT: kkq evicts, po evicts; DVE: XE, rec,
                # ---- final scale+bias; PE: qproj, VO -- so no PE
                # ---- instruction ever waits on the rbc bounce ----
                # next block's projection fills the PE while rbc is in flight
                if qb + 1 < NQB:
                    emit_qproj(qb + 1)

                rec = smalls.tile([1, FB], F32, tag="rec")
                nc.vector.reciprocal_approx_fast(out=rec, in_=den[0][:])
                rbc = psmm.tile([P, FB], F32, tag="mm", name=f"rbc{qb}")
                nc.tensor.matmul(
                    rbc, lhsT=ones1_sb[:], rhs=rec[:],
                    start=True, stop=True,
                )

                # ---- fused VO projection; po evicted to SBUF immediately so
                # ---- the PSUM slot never waits on the rbc broadcast ----
                for oc in range(CC):
                    po = psmm.tile([P, FB], F32, tag="mm", name=f"po{qb}_{oc}")
                    for ci, cc in enumerate(range(0, CC, 2)):
                        nc.tensor.matmul(
                            po, lhsT=n0_sb[:, cc:cc + 2, oc * P:(oc + 1) * P],
                            rhs=xe8[:, cc:cc + 2, :],
                            start=(ci == 0), stop=(ci == 1), perf_mode=DR,
                        )
                    yo = outs.tile([P, FB], F32, tag="yo", name=f"yo{qb}_{oc}")
                    nc.scalar.activation(out=yo, in_=po[:], func=AF.Copy)
                    ot = outs.tile([P, FB], F32, tag="ot")
                    nc.vector.tensor_mul(out=ot, in0=yo[:], in1=rbc[:])
                    nc.vector.tensor_scalar_add(out=ot, in0=ot[:], scalar1=b2_sb[:, oc:oc + 1])
                    nc.sync.dma_start(out=out_r[:, oc, qsl], in_=ot[:])

    nc.finalize()
    return nc


_NC_CACHE = {}


def _get_nc() -> bass.Bass:
    if "nc" not in _NC_CACHE:
        _NC_CACHE["nc"] = _build_bass()
    return _NC_CACHE["nc"]


def make_in_maps(q, Wq, bq, Wk, bk, Wv, bv, Wo, bo):
    f = np.float32
    f8 = ml_dtypes.float8_e4m3
    scale = f(C) ** f(-0.5)

    def q8(a):
        return np.ascontiguousarray(
            np.clip(np.asarray(a, f), -240, 240).astype(f8))

    Wq, Wk, Wv, Wo = (np.asarray(a, f) for a in (Wq, Wk, Wv, Wo))
    bq, bk, bv, bo = (np.asarray(a, f) for a in (bq, bk, bv, bo))
    M0p = ((Wk.T @ Wq) * (32.0 * scale))        # [o, c]
    m0 = q8(M0p.T.reshape(C, CC, P).transpose(1, 0, 2))  # [oc, c, o']
    n0 = q8((32.0 * (Wo @ Wv)).T)
    ones2 = np.full((P, 32), 2.0, f8)
    w_beta = (Wk.T @ bq) * scale
    b2 = (Wo @ bv + bo).astype(f)

    in_maps = []
    for core in range(N_CORES):
        b, half = core // 2, core % 2
        X = np.asarray(q[b], f).reshape(C, S)
        x8 = q8(X)
        in_maps.append({
            "x": x8,
            "xq": np.ascontiguousarray(x8[:, half * QH:(half + 1) * QH]),
            "xt": np.ascontiguousarray(x8.T),
            "m0": m0, "n0": n0, "ones2": ones2,
            "beta": (w_beta @ X).astype(f), "b2": b2,
        })
    return in_maps


def gather_out(per_core_outs):
    out = np.zeros((B, C, S), np.float32)
    for core in range(N_CORES):
        b, half = core // 2, core % 2
        out[b, :, half * QH:(half + 1) * QH] = per_core_outs[core]
    return out.reshape(B, C, H, W)


def kernel(q, Wq, bq, Wk, bk, Wv, bv, Wo, bo):
    nc = _get_nc()
    in_maps = make_in_maps(q, Wq, bq, Wk, bk, Wv, bv, Wo, bo)
    res = run_bass_kernel_spmd(nc, in_maps, core_ids=list(range(N_CORES)))
    return gather_out([res.results[i]["out"] for i in range(N_CORES)])


# revision 4
# speedup vs baseline: 1.1606x; 1.0050x over previous
"""Trainium2 Bass kernel for CrossAttention3D (single-head, 512-dim, 4x64x64).

Algebraic restructure (per batch, X = q[b] viewed (C, S)):
    Softmax logits are invariant to per-query additive constants, so with
    qp = Wq X + bq, kp = Wk X + bk:
        logits[q,k] = scale * qp_q . kp_k
                    = (scale Wk^T Wq X_q) . X_k + beta_k  (+ per-q const, dropped)
        beta_k      = scale * (Wk^T bq) . X_k             (+ const, dropped)
    and the V/O projections commute through the (linear) attention average:
        out = (Wo Wv)(X E)/den + (Wo bv + bo),  E = exp(logits), den = 1^T E.
    Only TWO device projections remain: the fused QK projection applied on
    the query side (2048 cols/core, not 4096) and the fused VO projection
    after attention. beta rides along as the exp's per-partition bias.

All heavy matmuls are fp8 e4m3 with DoubleRow perf mode (two 128-deep
k-tiles contracted per instruction; measured ~259ns per 512-row matmul =
2x bf16 FLOP rate, LdWeights fully hidden). Rescales keep operands in
e4m3's normal range: M0' = 32*scale*Wk^T Wq, N0' = 32*Wo Wv, exp scale=1/32,
XE evicted with scale 1/16, den "ones" = 2.0, so 32*(1/16)/2 = 1 net.

Schedule: per query block, sim leads and den/AV consume et one iteration
behind so the PE never waits on the exp. The denominator accumulates on the
DVE for iterations 0..13 (evicted to bf16 off the critical path) and the
last two et pairs close it directly in PSUM via DR matmuls. 1/den (approx
reciprocal on DVE) is broadcast to all partitions with a rank-1 f32 matmul
into PSUM -- no DRAM bounce -- and the VO-projection PSUM is evicted to
SBUF immediately so no PE instruction ever waits on the normalization.
Steady state measures ~216ns per matmul issue: the PE runs at ~99% of its
fp8 peak inside the attention loop.

Sharding: 8 cores = 4 batches x 2 query-halves, no collectives.
"""

import numpy as np
import ml_dtypes

import concourse.bass as bass
import concourse.bacc as bacc
import concourse.tile as tile
from concourse import mybir
from concourse.bass_utils import run_bass_kernel_spmd

AF = mybir.ActivationFunctionType
DR = mybir.MatmulPerfMode.DoubleRow
F32 = mybir.dt.float32
BF16 = mybir.dt.bfloat16
F8 = mybir.dt.float8e4

B, C, H, W = 4, 512, 64, 64
S = H * W            # 4096 tokens
P = 128              # partitions
CC = C // P          # 4 channel chunks
QH = S // 2          # 2048 queries per core
FB = 512             # query block
NQB = QH // FB       # 4 query blocks per core
NKC = S // P         # 32 key chunks
NI = NKC // 2        # 16 key-pair iterations
N_CORES = 8


def _build_bass() -> bass.Bass:
    nc = bacc.Bacc("TRN2", target_bir_lowering=False)

    x_d = nc.dram_tensor("x", [C, S], F8, kind="ExternalInput")     # keys, fp8
    xq_d = nc.dram_tensor("xq", [C, QH], F8, kind="ExternalInput")  # this core's queries
    xt_d = nc.dram_tensor("xt", [S, C], F8, kind="ExternalInput")   # X^T, fp8
    # (32*scale*Wk^T Wq) pre-chunked host-side: [oc, c, o'] contiguous per oc
    m0_d = nc.dram_tensor("m0", [CC, C, P], F8, kind="ExternalInput")
    n0_d = nc.dram_tensor("n0", [C, C], F8, kind="ExternalInput")   # (32*Wo@Wv)^T
    # all 2.0; padded to stride 16 to satisfy dual-fp8 Ldweights alignment
    ones_d = nc.dram_tensor("ones2", [P, 32], F8, kind="ExternalInput")
    beta_d = nc.dram_tensor("beta", [S], F32, kind="ExternalInput")
    b2_d = nc.dram_tensor("b2", [C], F32, kind="ExternalInput")
    out_d = nc.dram_tensor("out", [C, QH], BF16, kind="ExternalOutput")

    x_r = x_d[:, :].rearrange("(cc p) s -> p cc s", p=P)
    xq_r = xq_d[:, :].rearrange("(cc p) s -> p cc s", p=P)
    xt_r = xt_d[:, :].rearrange("(kc p) c -> p kc c", p=P)
    out_r = out_d[:, :].rearrange("(oc p) s -> p oc s", p=P)

    with tile.TileContext(nc) as tc:
        with (
            tc.tile_pool(name="consts", bufs=1) as consts,
            tc.tile_pool(name="kkqs", bufs=2) as kkqs,
            tc.tile_pool(name="ets", bufs=6) as ets,
            tc.tile_pool(name="esums", bufs=2) as esums,
            tc.tile_pool(name="xes", bufs=2) as xes,
            tc.tile_pool(name="smalls", bufs=4) as smalls,
            tc.tile_pool(name="outs", bufs=8) as outs,
            tc.tile_pool(name="psmm", bufs=4, space="PSUM") as psmm,   # 4 banks
            tc.tile_pool(name="psav", bufs=4, space="PSUM") as psav,   # 4 banks
        ):
            # ---- constants. x8/xt8/xq8 are split into per-chunk tiles so
            # ---- their DMAs stream without WAW completion waits; keys ride
            # ---- the sync queue, the transpose rides the gpsimd SWDGE
            # ---- queue, and ACT issues no DMAs (they would block evicts) ----
            m0_t = [consts.tile([P, CC, P], F8, tag=f"m0_{oc}", name=f"m0_{oc}")
                    for oc in range(CC)]
            nc.sync.dma_start(
                out=m0_t[0], in_=m0_d[0, :, :].rearrange("(cc p) o -> p cc o", p=P))
            xq8_t = [consts.tile([P, CC, FB], F8, tag=f"xq8_{i}", name=f"xq8_{i}")
                     for i in range(NQB)]
            nc.sync.dma_start(out=xq8_t[0], in_=xq_r[:, :, 0:FB])
            for oc in range(1, CC):
                nc.sync.dma_start(
                    out=m0_t[oc], in_=m0_d[oc, :, :].rearrange("(cc p) o -> p cc o", p=P))
            ones_sb = consts.tile([P, 1], BF16, tag="ones")
            nc.vector.memset(ones_sb, 2.0)
            ones8_sb = consts.tile([P, 2, 16], F8, tag="ones8")
            nc.sync.dma_start(out=ones8_sb, in_=ones_d[:, :])
            ones1_sb = consts.tile([1, P], F32, tag="ones1")
            nc.vector.memset(ones1_sb, 1.0)
            SQ = S // 4
            x8_t = [consts.tile([P, CC, SQ], F8, tag=f"x8_{i}", name=f"x8_{i}")
                    for i in range(4)]
            xt8_t = [consts.tile([P, NKC // 4, C], F8, tag=f"xt8_{i}", name=f"xt8_{i}")
                     for i in range(4)]
            nc.sync.dma_start(out=x8_t[0], in_=x_r[:, :, 0:SQ])
            beta_sb = consts.tile([P, NKC], F32, tag="beta")
            nc.sync.dma_start(out=beta_sb, in_=beta_d[:].rearrange("(kc p) -> p kc", p=P))
            nc.sync.dma_start(out=xt8_t[0], in_=xt_r[:, 0:NKC // 4, :])
            for c4 in range(1, 4):
                nc.sync.dma_start(out=x8_t[c4], in_=x_r[:, :, c4 * SQ:(c4 + 1) * SQ])
                klo = c4 * (NKC // 4)
                nc.sync.dma_start(out=xt8_t[c4], in_=xt_r[:, klo:klo + NKC // 4, :])
            for qb in range(1, NQB):
                nc.sync.dma_start(out=xq8_t[qb], in_=xq_r[:, :, qb * FB:(qb + 1) * FB])
            n0_sb = consts.tile([P, CC, C], F8, tag="n0")
            nc.sync.dma_start(out=n0_sb, in_=n0_d[:, :].rearrange("(cc p) o -> p cc o", p=P))
            b2_sb = consts.tile([P, CC], F32, tag="b2")
            nc.sync.dma_start(out=b2_sb, in_=b2_d[:].rearrange("(oc p) -> p oc", p=P))

            def x8_key(cc, k0):
                # sim lhsT [128, 2(cc pair), P] for keys [k0, k0+P)
                t = x8_t[k0 // SQ]
                off = k0 % SQ
                return t[:, cc:cc + 2, off:off + P]

            def xt8_key(kc, c4):
                # AV lhsT [128, 2(kc pair), P] for channel chunk c4
                t = xt8_t[kc // 8]
                return t[:, kc % 8:kc % 8 + 2, c4 * P:(c4 + 1) * P]

            kkq_tiles = {}

            def emit_qproj(qb):
                # fused QK projection of query block qb (PE + ACT evict)
                kkq = kkqs.tile([P, CC, FB], F8, tag="kkq", name=f"kkq{qb}")
                for oc in range(CC):
                    ps = psav.tile([P, FB], F32, tag="av")
                    for ci, cc in enumerate(range(0, CC, 2)):
                        nc.tensor.matmul(
                            ps, lhsT=m0_t[oc][:, cc:cc + 2, :],
                            rhs=xq8_t[qb][:, cc:cc + 2, :],
                            start=(ci == 0), stop=(ci == 1), perf_mode=DR,
                        )
                    nc.scalar.activation(out=kkq[:, oc, :], in_=ps[:], func=AF.Copy)
                kkq_tiles[qb] = kkq

            emit_qproj(0)

            for qb in range(NQB):
                qsl = slice(qb * FB, (qb + 1) * FB)
                kkq = kkq_tiles[qb]

                # ---- attention: sim leads; esum/AV consume et one iter behind ----
                avt = [psav.tile([P, FB], F32, tag="av", name=f"avt{qb}_{i}")
                       for i in range(CC)]
                esum = esums.tile([P, 2, FB], F32, tag="esum")
                ets_q = []

                def emit_sim(i, qb=qb, kkq=kkq):
                    simt = [psmm.tile([P, FB], F32, tag="mm", name=f"sim{qb}_{i}_{j}")
                            for j in range(2)]
                    for j in range(2):
                        k0 = (2 * i + j) * P
                        for ci, cc in enumerate(range(0, CC, 2)):
                            nc.tensor.matmul(
                                simt[j], lhsT=x8_key(cc, k0),
                                rhs=kkq[:, cc:cc + 2, :],
                                start=(ci == 0), stop=(ci == 1), perf_mode=DR,
                            )
                    et = ets.tile([P, 2, FB], F8, tag="et", name=f"et{qb}_{i}")
                    for j in range(2):
                        kc = 2 * i + j
                        nc.scalar.activation(
                            out=et[:, j, :], in_=simt[j][:], func=AF.Exp,
                            bias=beta_sb[:, kc:kc + 1], scale=1.0 / 32.0,
                        )
                    ets_q.append(et)

                den = [None]

                def emit_denav(i, qb=qb, avt=avt, esum=esum, ets_q=ets_q):
                    et = ets_q[i]
                    # denominator partials: iters 0..13 accumulate on the DVE
                    # (off the critical path); the last two go straight into
                    # the den PSUM with DR matmuls so den closes ~1us after
                    # the final exp
                    if i == 0:
                        nc.vector.tensor_scalar_add(out=esum, in0=et[:, :, :], scalar1=0.0)
                    elif i <= NI - 3:
                        nc.vector.tensor_add(out=esum, in0=esum[:, :, :], in1=et[:, :, :])
                    else:
                        if i == NI - 2:
                            den[0] = psmm.tile([1, FB], F32, tag="mm", name=f"den{qb}")
                        nc.tensor.matmul(
                            den[0], lhsT=ones8_sb[:, :, 0:1], rhs=et[:, :, :],
                            start=(i == NI - 2), stop=False, perf_mode=DR,
                        )
                    for c4 in range(CC):
                        nc.tensor.matmul(
                            avt[c4], lhsT=xt8_key(2 * i, c4),
                            rhs=et[:, :, :],
                            start=(i == 0), stop=(i == NI - 1), perf_mode=DR,
                        )

                for i in range(NI):
                    emit_sim(i)
                    if i > 0:
                        emit_denav(i - 1)
                    if i == NI - 1:
                        # esbA (iters 0..13) evicts early, off the tail path
                        esb = smalls.tile([P, 2, FB], BF16, tag="esb")
                        nc.scalar.activation(out=esb, in_=esum[:, :, :], func=AF.Copy)
                # last iteration: AV matmuls + den DR matmul, then XE evicts
                et15 = ets_q[NI - 1]
                for c4 in range(CC):
                    nc.tensor.matmul(
                        avt[c4], lhsT=xt8_key(2 * (NI - 1), c4),
                        rhs=et15[:, :, :],
                        start=False, stop=True, perf_mode=DR,
                    )
                nc.tensor.matmul(
                    den[0], lhsT=ones8_sb[:, :, 0:1], rhs=et15[:, :, :],
                    start=False, stop=False, perf_mode=DR,
                )
                nc.tensor.matmul(den[0], lhsT=ones_sb[:], rhs=esb[:, 0, :], start=False, stop=False)
                nc.tensor.matmul(den[0], lhsT=ones_sb[:], rhs=esb[:, 1, :], start=False, stop=True)
                xe8 = xes.tile([P, CC, FB], F8, tag="xe8")
                last = qb == NQB - 1
                if last:
                    # tail: reciprocal leads the DVE queue so the broadcast
                    # matmul (and thus the output chain) starts ~2us earlier
                    rec = smalls.tile([1, FB], F32, tag="rec", name="rec_last")
                    nc.vector.reciprocal_approx_fast(out=rec, in_=den[0][:])
                for c4 in range(CC):
                    nc.vector.tensor_scalar_mul(
                        out=xe8[:, c4, :], in0=avt[c4][:], scalar1=1.0 / 16.0)

                # ---- boundary. ACT: kkq evicts, po evicts; DVE: XE, rec,
                # ---- final scale+bias; PE: qproj, VO -- so no PE
                # ---- instruction ever waits on the rbc bounce ----
                # next block's projection fills the PE while rbc is in flight
                if qb + 1 < NQB:
                    emit_qproj(qb + 1)

                if not last:
                    rec = smalls.tile([1, FB], F32, tag="rec")
                    nc.vector.reciprocal_approx_fast(out=rec, in_=den[0][:])
                rbc = psmm.tile([P, FB], F32, tag="mm", name=f"rbc{qb}")
                nc.tensor.matmul(
                    rbc, lhsT=ones1_sb[:], rhs=rec[:],
                    start=True, stop=True,
                )

                # ---- fused VO projection; po evicted to SBUF immediately so
                # ---- the PSUM slot never waits on the rbc broadcast ----
                for oc in range(CC):
                    po = psmm.tile([P, FB], F32, tag="mm", name=f"po{qb}_{oc}")
                    for ci, cc in enumerate(range(0, CC, 2)):
                        nc.tensor.matmul(
                            po, lhsT=n0_sb[:, cc:cc + 2, oc * P:(oc + 1) * P],
                            rhs=xe8[:, cc:cc + 2, :],
                            start=(ci == 0), stop=(ci == 1), perf_mode=DR,
                        )
                    yo = outs.tile([P, FB], F32, tag="yo", name=f"yo{qb}_{oc}")
                    nc.scalar.activation(out=yo, in_=po[:], func=AF.Copy)
                    ot = outs.tile([P, FB], BF16, tag="ot")
                    nc.vector.tensor_mul(out=ot, in0=yo[:], in1=rbc[:])
                    nc.vector.tensor_scalar_add(out=ot, in0=ot[:], scalar1=b2_sb[:, oc:oc + 1])
                    # last block: alternate out-DMA queues (ACT is idle then)
                    eng = nc.scalar if (last and oc % 2 == 1) else nc.sync
                    eng.dma_start(out=out_r[:, oc, qsl], in_=ot[:])

    nc.finalize()
    return nc


_NC_CACHE = {}


def _get_nc() -> bass.Bass:
    if "nc" not in _NC_CACHE:
        _NC_CACHE["nc"] = _build_bass()
    return _NC_CACHE["nc"]


def make_in_maps(q, Wq, bq, Wk, bk, Wv, bv, Wo, bo):
    f = np.float32
    f8 = ml_dtypes.float8_e4m3
    scale = f(C) ** f(-0.5)

    def q8(a):
        return np.ascontiguousarray(
            np.clip(np.asarray(a, f), -240, 240).astype(f8))

    Wq, Wk, Wv, Wo = (np.asarray(a, f) for a in (Wq, Wk, Wv, Wo))
    bq, bk, bv, bo = (np.asarray(a, f) for a in (bq, bk, bv, bo))
    M0p = ((Wk.T @ Wq) * (32.0 * scale))        # [o, c]
    m0 = q8(M0p.T.reshape(C, CC, P).transpose(1, 0, 2))  # [oc, c, o']
    n0 = q8((32.0 * (Wo @ Wv)).T)
    ones2 = np.full((P, 32), 2.0, f8)
    w_beta = (Wk.T @ bq) * scale
    b2 = (Wo @ bv + bo).astype(f)

    in_maps = []
    for core in range(N_CORES):
        b, half = core // 2, core % 2
        X = np.asarray(q[b], f).reshape(C, S)
        x8 = q8(X)
        in_maps.append({
            "x": x8,
            "xq": np.ascontiguousarray(x8[:, half * QH:(half + 1) * QH]),
            "xt": np.ascontiguousarray(x8.T),
            "m0": m0, "n0": n0, "ones2": ones2,
            "beta": (w_beta @ X).astype(f), "b2": b2,
        })
    return in_maps


def gather_out(per_core_outs):
    out = np.zeros((B, C, S), np.float32)
    for core in range(N_CORES):
        b, half = core // 2, core % 2
        out[b, :, half * QH:(half + 1) * QH] = np.asarray(
            per_core_outs[core]).astype(np.float32)
    return out.reshape(B, C, H, W)


def kernel(q, Wq, bq, Wk, bk, Wv, bv, Wo, bo):
    nc = _get_nc()
    in_maps = make_in_maps(q, Wq, bq, Wk, bk, Wv, bv, Wo, bo)
    res = run_bass_kernel_spmd(nc, in_maps, core_ids=list(range(N_CORES)))
    return gather_out([res.results[i]["out"] for i in range(N_CORES)])
